# revision 68
# baseline (speedup 1.0000x reference)
"""GQA attention (B=2,T=2048,D=2048,H=16,KV=4,HD=128, causal+RoPE) on 8 trn2 cores.

Sharding: 4-way head tensor-parallel x 2-way batch data-parallel.
Core c: batch b=c//4, TP shard s=c%4 -> q heads [4s..4s+3], kv head s.

Transpose-free attention: scores are computed transposed (S^T: keys on
partitions, queries on the free axis), so exp(S^T) lands in SBUF already in
the layout the PV matmul wants as its moving operand; PV with lhsT=V produces
O^T directly. The softmax denominator comes from a ones-column matmul over
the same P^T tiles; 1/l is computed with the fast approximate reciprocal on
DVE and broadcast across partitions on the (otherwise idle) GPSIMD engine.
Causality is exact at 128-column granularity (diagonal blocks use a single
triangular mask tile and trimmed matmul/exp widths). V is produced via a
W-stationary V^T chain plus PE transposes, fused into the projection pass.
Input DMAs are split across the two hardware DGE queues (sync + scalar) in
few large, consumption-ordered pieces; projections, attention groups, and
output projection are interleaved so the tensor engine never drains.

Host side: RoPE even/odd gather is folded into column permutations of Wq/Wk,
the 1/sqrt(HD) score scale is folded into Wq, and the 4 TP partial outputs
per batch are summed at the end.
"""

import math
import os
import numpy as np

try:
    import concourse.bass as bass
except ImportError:  # pragma: no cover
    import sys

    sys.path.insert(0, "/opt/trn_rl_repo")
    import concourse.bass as bass

import concourse.mybir as mybir
import concourse.bacc as bacc
from concourse import bass_utils
from concourse.tile import TileContext
from contextlib import ExitStack
from ml_dtypes import bfloat16

B, T, D = 2, 2048, 2048
H, KV, HD = 16, 4, 128
TP = 4  # head-TP ways
NH = H // TP  # q heads per core = 4
NKB = D // 128  # 16 contraction blocks
NTC = T // 512  # 4 free-dim chunks
NTB = T // 128  # 16 token blocks
NG = T // 512  # 4 query groups
SCALE = 1.0 / math.sqrt(HD)
F32 = mybir.dt.float32
BF16 = mybir.dt.bfloat16
EXP = mybir.ActivationFunctionType.Exp
MASK_VAL = -1e9

_program = None
_last_results = None
last_exec_time_ns = None


def _build_program():
    global _program
    if _program is not None:
        return _program

    nc = bacc.Bacc(
        "TRN2",
        target_bir_lowering=False,
        debug=False,
        enable_asserts=False,
        num_devices=8,
    )
    xT_d = nc.dram_tensor("xT", [NTC, 128, NKB, 512], BF16, kind="ExternalInput").ap()
    wq_d = nc.dram_tensor("Wq", [128, NKB * NH * 128], BF16, kind="ExternalInput").ap()
    wk_d = nc.dram_tensor("Wk", [128, NKB * 128], BF16, kind="ExternalInput").ap()
    wv_d = nc.dram_tensor("Wv", [128, NKB * 128], BF16, kind="ExternalInput").ap()
    wo_d = nc.dram_tensor("Wo", [128, NH * D], BF16, kind="ExternalInput").ap()
    cos_d = nc.dram_tensor("cosq", [128, T], BF16, kind="ExternalInput").ap()
    sin_d = nc.dram_tensor("sinq", [128, T], BF16, kind="ExternalInput").ap()
    tri_d = nc.dram_tensor("tri", [128, 128], F32, kind="ExternalInput").ap()
    id_d = nc.dram_tensor("ident", [128, 128], BF16, kind="ExternalInput").ap()
    y_d = nc.dram_tensor("y", [T, D], BF16, kind="ExternalOutput").ap()

    with TileContext(nc) as tc, ExitStack() as ctx:
        big = ctx.enter_context(tc.tile_pool(name="big", bufs=1))
        psA = ctx.enter_context(tc.tile_pool(name="psA", bufs=4, space="PSUM"))
        psOT = ctx.enter_context(tc.tile_pool(name="psOT", bufs=2, space="PSUM"))
        psL = ctx.enter_context(tc.tile_pool(name="psL", bufs=1, space="PSUM"))
        psV = ctx.enter_context(tc.tile_pool(name="psV", bufs=1, space="PSUM"))
        rtmp = ctx.enter_context(tc.tile_pool(name="rtmp", bufs=2))
        ptp = ctx.enter_context(tc.tile_pool(name="ptp", bufs=20))
        rlp = ctx.enter_context(tc.tile_pool(name="rlp", bufs=2))
        ypool = ctx.enter_context(tc.tile_pool(name="ypool", bufs=4))

        xT = big.tile([128, NKB, T], BF16, tag="xT")
        wq = big.tile([128, NKB, NH, 128], BF16, tag="wq")
        wk = big.tile([128, NKB, 128], BF16, tag="wk")
        wv = big.tile([128, NKB, 128], BF16, tag="wv")
        wo = big.tile([128, NH, D], BF16, tag="wo")
        cos = big.tile([128, T], BF16, tag="cos")
        sin = big.tile([128, T], BF16, tag="sin")
        tri = big.tile([128, 128], F32, tag="tri")
        qT = big.tile([128, NH, T], BF16, tag="qT")
        kT = big.tile([128, T], BF16, tag="kT")
        VT = big.tile([128, T], BF16, tag="VT")
        V = big.tile([128, NTB, 128], BF16, tag="V")
        OT = big.tile([128, NH, T], BF16, tag="OT")
        ones1 = big.tile([128, 1], BF16, tag="ones1")
        ident = big.tile([128, 128], BF16, tag="ident")

        # ---- loads: few big DMAs (dispatch is ~0.6us of issuing-engine time),
        # bytes split across the two hardware DGE queues (sync / scalar),
        # ordered so qk_chunk(t) unblocks in sequence ----
        nc.vector.memset(ones1[:], 1.0)

        def xT_chunk_load(eng, tcc, jlo, jhi):
            eng.dma_start(
                out=xT[:, jlo:jhi, tcc * 512 : (tcc + 1) * 512],
                in_=xT_d[tcc, :, jlo:jhi, :],
            )

        # chunk 0 in j-group pieces so the first projection chains start early
        for j2 in range(0, NKB, 2):
            xT_chunk_load(nc.sync, 0, j2, j2 + 2)
            nc.scalar.dma_start(
                out=wq[:, j2 : j2 + 2, :, :],
                in_=wq_d[:, j2 * 512 : (j2 + 2) * 512],
            )
        nc.scalar.dma_start(out=wk[:], in_=wk_d[:])
        nc.scalar.dma_start(out=wv[:], in_=wv_d[:])
        nc.sync.dma_start(out=cos[:], in_=cos_d[:])
        nc.scalar.dma_start(out=sin[:], in_=sin_d[:])
        nc.sync.dma_start(out=tri[:], in_=tri_d[:])
        nc.sync.dma_start(out=ident[:], in_=id_d[:])
        xT_chunk_load(nc.scalar, 1, 0, NKB)
        xT_chunk_load(nc.sync, 2, 0, NKB)
        xT_chunk_load(nc.scalar, 3, 0, NKB)
        nc.sync.dma_start(out=wo[:], in_=wo_d[:])

        # ---- projections with fused RoPE (head dim on partitions) ----
        # scalar (idle in the projection phase) drains the half-rotated q/k
        # to bf16 SBUF, so the rot-multiply is a single full-width 2x-rate
        # bf16 DVE op instead of two half-partition f32 ops
        def rope(ps, dst, sl, use_scalar=True):
            t2 = rtmp.tile([128, 512], BF16, tag="t2")
            if use_scalar:
                sbr = rtmp.tile([128, 512], BF16, tag="sb")
                nc.scalar.copy(sbr[0:64, :], ps[64:128, :])
                nc.scalar.copy(sbr[64:128, :], ps[0:64, :])
                nc.vector.tensor_mul(t2[:], sbr[:], sin[:, sl])
            else:
                nc.vector.tensor_mul(t2[0:64, :], ps[64:128, :], sin[0:64, sl])
                nc.vector.tensor_mul(t2[64:128, :], ps[0:64, :], sin[64:128, sl])
            t1 = rtmp.tile([128, 512], BF16, tag="t1")
            nc.vector.tensor_mul(t1[:], ps, cos[:, sl])
            nc.vector.tensor_add(dst, t1[:], t2[:])

        def qk_chunk(tcc):
            sl = slice(tcc * 512, (tcc + 1) * 512)
            for h in range(NH):
                pst = psA.tile([128, 512], F32, tag="psA")
                for j in range(NKB):
                    nc.tensor.matmul(
                        pst[:],
                        lhsT=wq[:, j, h, :],
                        rhs=xT[:, j, sl],
                        start=(j == 0),
                        stop=(j == NKB - 1),
                    )
                rope(pst, qT[:, h, sl], sl, use_scalar=(tcc > 0))
            pst = psA.tile([128, 512], F32, tag="psA")
            for j in range(NKB):
                nc.tensor.matmul(
                    pst[:],
                    lhsT=wk[:, j, :],
                    rhs=xT[:, j, sl],
                    start=(j == 0),
                    stop=(j == NKB - 1),
                )
            rope(pst, kT[:, sl], sl, use_scalar=(tcc > 0))
            # V^T chain (no rope), then transpose 128-blocks into V layout
            pst = psOT.tile([128, 512], F32, tag="ot")
            for j in range(NKB):
                nc.tensor.matmul(
                    pst[:],
                    lhsT=wv[:, j, :],
                    rhs=xT[:, j, sl],
                    start=(j == 0),
                    stop=(j == NKB - 1),
                )
            if tcc > 0:
                nc.scalar.copy(VT[:, sl], pst[:])
            else:
                nc.vector.tensor_copy(VT[:, sl], pst[:])
            for tb in range(4 * tcc, 4 * tcc + 4):
                vtp = psV.tile([128, 128], BF16, tag="vtp")
                nc.tensor.transpose(
                    vtp[:], VT[:, tb * 128 : (tb + 1) * 128], ident[:]
                )
                if tcc > 0:
                    nc.scalar.copy(V[:, tb, :], vtp[:])
                else:
                    nc.vector.tensor_copy(V[:, tb, :], vtp[:])

        # ---- causal attention for one 512-query group ----
        def attn_group(g):
            nb = 4 * (g + 1)
            q0 = g * 512
            for hp in range(2):
                h0, h1 = 2 * hp, 2 * hp + 1
                ot0 = psOT.tile([128, 512], F32, tag="ot")
                ot1 = psOT.tile([128, 512], F32, tag="ot")
                lt = psL.tile([64, 512], F32, tag="l")
                pts = []
                for b in range(nb):
                    dj = b - 4 * g
                    off = dj * 128 if dj >= 0 else 0
                    st0 = psA.tile([128, 512], F32, tag="psA")
                    st1 = psA.tile([128, 512], F32, tag="psA")
                    kblk = kT[:, b * 128 : (b + 1) * 128]
                    nc.tensor.matmul(
                        st0[:, off:512],
                        lhsT=kblk,
                        rhs=qT[:, h0, q0 + off : q0 + 512],
                        start=True,
                        stop=True,
                    )
                    nc.tensor.matmul(
                        st1[:, off:512],
                        lhsT=kblk,
                        rhs=qT[:, h1, q0 + off : q0 + 512],
                        start=True,
                        stop=True,
                    )
                    if dj >= 0:
                        nc.vector.tensor_add(
                            st0[:, off : off + 128], st0[:, off : off + 128], tri[:]
                        )
                        nc.vector.tensor_add(
                            st1[:, off : off + 128], st1[:, off : off + 128], tri[:]
                        )
                    pt0 = ptp.tile([128, 512], BF16, tag="pt")
                    pt1 = ptp.tile([128, 512], BF16, tag="pt")
                    nc.scalar.activation(pt0[:, off:512], st0[:, off:512], EXP)
                    nc.scalar.activation(pt1[:, off:512], st1[:, off:512], EXP)
                    pts.append((pt0, pt1, off))
                for b in range(nb):
                    pt0, pt1, off = pts[b]
                    first, last = (b == 0), (b == nb - 1)
                    vblk = V[:, b, :]
                    nc.tensor.matmul(
                        ot0[:, off:512],
                        lhsT=vblk,
                        rhs=pt0[:, off:512],
                        start=first,
                        stop=last,
                    )
                    nc.tensor.matmul(
                        ot1[:, off:512],
                        lhsT=vblk,
                        rhs=pt1[:, off:512],
                        start=first,
                        stop=last,
                    )
                    nc.tensor.matmul(
                        lt[0:1, off:512],
                        lhsT=ones1[:],
                        rhs=pt0[:, off:512],
                        start=first,
                        stop=last,
                    )
                    nc.tensor.matmul(
                        lt[32:33, off:512],
                        lhsT=ones1[:],
                        rhs=pt1[:, off:512],
                        start=first,
                        stop=last,
                    )
                lsb0 = rlp.tile([1, 512], F32, tag="lsb")
                lsb1 = rlp.tile([1, 512], F32, tag="lsb")
                nc.vector.tensor_copy(lsb0[:], lt[0:1, :])
                nc.vector.tensor_copy(lsb1[:], lt[32:33, :])
                rl0 = rlp.tile([1, 512], F32, tag="rl")
                rl1 = rlp.tile([1, 512], F32, tag="rl")
                nc.vector.reciprocal_approx_fast(out=rl0[:], in_=lsb0[:])
                nc.vector.reciprocal_approx_fast(out=rl1[:], in_=lsb1[:])
                rb0 = rlp.tile([1, 512], BF16, tag="rb")
                rb1 = rlp.tile([1, 512], BF16, tag="rb")
                nc.vector.tensor_copy(rb0[:], rl0[:])
                nc.vector.tensor_copy(rb1[:], rl1[:])
                bc0 = rlp.tile([128, 512], BF16, tag="bc")
                nc.gpsimd.partition_broadcast(bc0[:], rb0[:])
                bc1 = rlp.tile([128, 512], BF16, tag="bc")
                nc.gpsimd.partition_broadcast(bc1[:], rb1[:])
                nc.vector.tensor_mul(OT[:, h0, q0 : q0 + 512], ot0[:], bc0[:])
                nc.vector.tensor_mul(OT[:, h1, q0 : q0 + 512], ot1[:], bc1[:])

        # ---- output projection for one 512-query group ----
        def wo_group(g):
            for qc in range(4):
                r0 = g * 512 + qc * 128
                for dc in range(4):
                    ypt = psA.tile([128, 512], F32, tag="psA")
                    for h in range(NH):
                        nc.tensor.matmul(
                            ypt[:],
                            lhsT=OT[:, h, r0 : r0 + 128],
                            rhs=wo[:, h, dc * 512 : (dc + 1) * 512],
                            start=(h == 0),
                            stop=(h == NH - 1),
                        )
                    ysb = ypool.tile([128, 512], BF16, tag="y")
                    if g == 3 and dc % 2 == 1:
                        nc.scalar.copy(ysb[:], ypt[:])
                        deng = nc.scalar
                    else:
                        nc.vector.tensor_copy(ysb[:], ypt[:])
                        deng = nc.sync
                    deng.dma_start(
                        out=y_d[r0 : r0 + 128, dc * 512 : (dc + 1) * 512],
                        in_=ysb[:],
                    )

        qk_chunk(0)
        attn_group(0)
        qk_chunk(1)
        attn_group(1)
        wo_group(0)
        qk_chunk(2)
        attn_group(2)
        wo_group(1)
        qk_chunk(3)
        attn_group(3)
        wo_group(2)
        wo_group(3)

    nc.compile()
    _program = nc
    return nc


def _host_prep(x, Wq, Wk, Wv, Wo):
    x = np.asarray(x, dtype=np.float32)
    Wq = np.asarray(Wq, dtype=np.float32)
    Wk = np.asarray(Wk, dtype=np.float32)
    Wv = np.asarray(Wv, dtype=np.float32)
    Wo = np.asarray(Wo, dtype=np.float32)

    # RoPE even/odd gather folded into weight column permutation (per head);
    # 1/sqrt(HD) score scale folded into Wq.
    perm128 = np.r_[np.arange(0, 128, 2), np.arange(1, 128, 2)]
    permq = np.concatenate([hb * 128 + perm128 for hb in range(H)])
    permk = np.concatenate([hb * 128 + perm128 for hb in range(KV)])
    Wq_p = (Wq * SCALE)[:, permq]
    Wk_p = Wk[:, permk]

    pos = np.arange(T, dtype=np.float64)
    inv_freq = 1.0 / (10000.0 ** (np.arange(0, HD, 2, dtype=np.float64) / HD))
    ang = np.einsum("t,f->tf", pos, inv_freq)  # [T, 64]
    cos_t = np.cos(ang).T.astype(np.float32)  # [64, T]
    sin_t = np.sin(ang).T.astype(np.float32)
    cosB = np.concatenate([cos_t, cos_t], axis=0).astype(bfloat16)  # [128, T]
    sinB = np.concatenate([-sin_t, sin_t], axis=0).astype(bfloat16)

    r = np.arange(128)
    tri = np.where(r[:, None] > r[None, :], MASK_VAL, 0.0).astype(np.float32)

    in_maps = []
    for c in range(8):
        b, s = c // 4, c % 4
        wq_c = Wq_p[:, s * 512 : (s + 1) * 512]
        wq_c = (
            wq_c.reshape(NKB, 128, NH, 128)
            .transpose(1, 0, 2, 3)
            .reshape(128, NKB * NH * 128)
        )
        wk_c = Wk_p[:, s * 128 : (s + 1) * 128]
        wk_c = wk_c.reshape(NKB, 128, 128).transpose(1, 0, 2).reshape(128, NKB * 128)
        wv_c = Wv[:, s * 128 : (s + 1) * 128]
        wv_c = wv_c.reshape(NKB, 128, 128).transpose(1, 0, 2).reshape(128, NKB * 128)
        wo_c = Wo[s * 512 : (s + 1) * 512, :]
        wo_c = wo_c.reshape(NH, 128, D).transpose(1, 0, 2).reshape(128, NH * D)
        xTc = (
            np.ascontiguousarray(x[b].T)
            .reshape(NKB, 128, NTC, 512)
            .transpose(2, 1, 0, 3)
        )
        in_maps.append(
            {
                "xT": np.ascontiguousarray(xTc).astype(bfloat16),
                "Wq": np.ascontiguousarray(wq_c).astype(bfloat16),
                "Wk": np.ascontiguousarray(wk_c).astype(bfloat16),
                "Wv": np.ascontiguousarray(wv_c).astype(bfloat16),
                "Wo": np.ascontiguousarray(wo_c).astype(bfloat16),
                "cosq": cosB,
                "sinq": sinB,
                "tri": tri,
                "ident": np.eye(128, dtype=bfloat16),
            }
        )
    return in_maps


def _ensure_ntff_hook():
    """The agent image's antenv lacks axon_hooks, so boot() skips installing
    the NTFF profile hook. Recreate the module and install the hook."""
    import sys
    import types

    try:
        from antenv.axon_hooks import get_axon_ntff_profile_hook  # noqa: F401

        return True
    except ImportError:
        pass
    try:
        import antenv
        from trn_agent_boot.trn_boot import _ntff_profile_via_ctypes

        hook = _ntff_profile_via_ctypes("/opt/axon/libaxon_pjrt.so")
        if hook is None:
            return False
        mod = types.ModuleType("antenv.axon_hooks")
        mod._hook = hook
        mod.set_axon_ntff_profile_hook = lambda h: setattr(mod, "_hook", h)
        mod.get_axon_ntff_profile_hook = lambda: mod._hook
        sys.modules["antenv.axon_hooks"] = mod
        antenv.axon_hooks = mod
        bass_utils.upload_artifacts = lambda d: d
        return True
    except Exception:
        return False


def kernel(x, Wq, Wk, Wv, Wo):
    global _last_results, last_exec_time_ns
    nc = _build_program()
    in_maps = _host_prep(x, Wq, Wk, Wv, Wo)
    trace = bool(int(os.environ.get("KERNEL_TRACE", "0")))
    tmpdir = None
    if trace:
        trace = _ensure_ntff_hook()
        if trace:
            tmpdir = os.environ.get("KERNEL_TRACE_DIR") or None
    res = bass_utils.run_bass_kernel_spmd(
        nc, in_maps, core_ids=list(range(8)), trace=trace, tmpdir=tmpdir
    )
    _last_results = res
    last_exec_time_ns = res.exec_time_ns
    out = np.empty((B, T, D), dtype=np.float32)
    for b in range(B):
        out[b] = sum(
            res.results[4 * b + s]["y"].astype(np.float32) for s in range(TP)
        )
    return out
